# revision 30
# baseline (speedup 1.0000x reference)
"""Trainium2 Bass kernel for nn_GCGNet (2x GCN drug encoder + dense fusion head).

Strategy (8 NeuronCores, SPMD):
  - Shard target nodes / graphs: core c owns nodes [c*3200, (c+1)*3200) and
    graphs [c*64, (c+1)*64).
  - GCNConv reformulation: with deg = indegree+1, dinv = deg^-1/2,
    y = dinv * x (row scale), each layer's pre-activation out rows are
        out = dinv * (A @ y + y) @ W + b
    so the gather tables hold y (bf16, 256B-aligned rows in DRAM) and the
    edge coefficient vanishes: the scatter matrix is pure 0/1.
  - Aggregation A @ y: fixed-degree main slots (16 per node) gathered with
    dma_gather, reduced on the TensorEngine with a constant one-hot
    P16 [128,16->8] (slot partition p feeds node p//16), plus a self group
    (identity) and a few shipped one-hot overflow blocks for deg > 16.
  - Bias folding: s' = [scale*s, col] @ [W; b] with (scale, col) =
    (dinv^2, dinv) for layers 1-2 (fuses the next table's y scaling) and
    (dinv, 1) for layer 3.
  - Layers 1-2 write node-major table slabs -> AllGather across the 8 cores.
    Layer 3 emits feature-major h3T for segment-max pooling; the whole dense
    fusion head is graph-local per core, output [2, 64] per core.
"""

import numpy as np
import ml_dtypes

BF16 = ml_dtypes.bfloat16

# ---- problem constants (hardcoded per the task contract) -------------------
N_NODES = 25600
N_EDGES = 409600
N_GRAPHS = 512
FXD = 78
FXT = 954
ODIM = 128
N_OUT = 2

NCORES = 8
NLOC = N_NODES // NCORES          # 3200 nodes per core
GLOC = N_GRAPHS // NCORES         # 64 graphs per core
NPG = N_NODES // N_GRAPHS         # 50 nodes per graph
TILE = 128
NT = NLOC // TILE                 # 25 node tiles per core

D_MAIN = 16                       # fixed-degree main slots per node
ZROW = N_NODES                    # index of the all-zero table row
TAB_ROWS = N_NODES + TILE         # 25728
RE12 = 128                        # table row elems (bf16) for layers 1,2 (F=78)
RE3 = 256                         # table row elems (bf16) for layer 3 (F=156)

F1, F2, F3 = FXD, 2 * FXD, 4 * FXD   # 78, 156, 312


# ===========================================================================
# Host-side prep: sharding + index structures (pure integer work)
# ===========================================================================

def _prep_encoder(edge_index):
    """Build per-core gather/scatter index structures for one edge set.

    Returns dict with:
      deg        [N_NODES] f32    (indegree + 1)
      slots      [NCORES, NLOC, D_MAIN] int32   (src ids, ZROW = pad)
      ovf_src    list-of-lists per (core, tile): np arrays of src ids
      ovf_tloc   same shape: target offset within the 128-node tile
    """
    src = edge_index[0].astype(np.int64)
    tgt = edge_index[1].astype(np.int64)
    counts = np.bincount(tgt, minlength=N_NODES)
    deg = counts.astype(np.float32) + 1.0

    order = np.argsort(tgt, kind="stable")
    ssrc = src[order]
    stgt = tgt[order]
    rowptr = np.zeros(N_NODES + 1, np.int64)
    rowptr[1:] = np.cumsum(counts)
    pos = np.arange(N_EDGES, dtype=np.int64) - rowptr[stgt]

    main_mask = pos < D_MAIN
    slots = np.full((N_NODES, D_MAIN), ZROW, np.int32)
    slots[stgt[main_mask], pos[main_mask]] = ssrc[main_mask]
    slots = slots.reshape(NCORES, NLOC, D_MAIN)

    o_src = ssrc[~main_mask]
    o_tgt = stgt[~main_mask]
    ovf_src = [[None] * NT for _ in range(NCORES)]
    ovf_tloc = [[None] * NT for _ in range(NCORES)]
    tile_id = o_tgt // TILE
    for c in range(NCORES):
        for t in range(NT):
            m = tile_id == c * NT + t
            ovf_src[c][t] = o_src[m]
            ovf_tloc[c][t] = (o_tgt[m] - (c * NT + t) * TILE).astype(np.int64)
    return dict(deg=deg, slots=slots, ovf_src=ovf_src, ovf_tloc=ovf_tloc)


def _max_ovf_blocks(preps):
    m = 1
    for pr in preps:
        for c in range(NCORES):
            for t in range(NT):
                m = max(m, -(-len(pr["ovf_src"][c][t]) // TILE))
    return m


def _build_core_arrays(pr, m_ovf):
    """Per-core flat gather-index arrays + overflow one-hot blocks.

    idx   [NCORES, NT*(17+m_ovf)*128] int32  slot order:
          tile t, group g (0..15 main, 16 self, 17.. ovf), partition p
    povf  [NCORES, NT, m_ovf, 128, 128] bf16 one-hot (slot p, local node)
    """
    gpt = 17 + m_ovf
    idx = np.full((NCORES, NT * gpt * 128), ZROW, np.int32)
    povf = np.zeros((NCORES, NT, m_ovf, TILE, TILE), dtype=BF16)
    slots = pr["slots"]
    for c in range(NCORES):
        a = idx[c].reshape(NT, gpt, 128)
        for t in range(NT):
            base = t * TILE
            # main groups: group index qg = q*4+g covers nodes
            # base + q*32 + p//4 with edge slot j = g*4 + p%4
            qg_idx = np.arange(16)
            p_idx = np.arange(128)
            q = qg_idx[:, None] // 4
            g = qg_idx[:, None] % 4
            nodes = base + q * 32 + p_idx[None, :] // 4        # [16,128]
            js = g * 4 + p_idx[None, :] % 4                    # [16,128]
            a[t, :16, :] = slots[c][nodes, js]
            # self group: absolute node id
            a[t, 16, :] = c * NLOC + base + p_idx
            # overflow groups
            osrc = pr["ovf_src"][c][t]
            otl = pr["ovf_tloc"][c][t]
            for b in range(m_ovf):
                lo, hi = b * TILE, min((b + 1) * TILE, len(osrc))
                if lo >= len(osrc):
                    break
                n = hi - lo
                a[t, 17 + b, :n] = osrc[lo:hi]
                povf[c, t, b, np.arange(n), otl[lo:hi]] = 1.0
    return idx, povf


def _idx_sbuf_layout(idx_flat):
    """int32 flat slot list -> int16 [128, len/16] SBUF image.

    dma_gather reads index i from partition i%16, column i//16; replicate
    across the 8 groups of 16 partitions for the 8 Q7 cores.
    """
    n = idx_flat.shape[0]
    assert n % 16 == 0
    a = idx_flat.reshape(n // 16, 16).T.astype(np.int16)  # [16, n/16]
    return np.tile(a, (8, 1))                              # [128, n/16]


def _wb(W, b):
    """Stack [W; b] -> [in+1, out] f32."""
    return np.concatenate([W, b[None, :]], axis=0).astype(np.float32)


def _deg_tiled(deg, ntiles):
    """deg [ntiles*128] -> [128, ntiles] (partition p, col t = deg[t*128+p])."""
    return deg.reshape(ntiles, 128).T.copy()


def prep_all(x1, edge_index1, x2, edge_index2, cell, params):
    """All host-side prep. Returns (meta, per_core_inputs list)."""
    p1 = _prep_encoder(np.asarray(edge_index1))
    p2 = _prep_encoder(np.asarray(edge_index2))
    m_ovf = _max_ovf_blocks([p1, p2])
    idx1, povf1 = _build_core_arrays(p1, m_ovf)
    idx2, povf2 = _build_core_arrays(p2, m_ovf)

    pm = params
    weights = {
        "w1": _wb(pm["conv1_W"], pm["conv1_b"]),
        "w2": _wb(pm["conv2_W"], pm["conv2_b"]),
        "w3": _wb(pm["conv3_W"], pm["conv3_b"]),
        "wg1": _wb(pm["fc_g1_W"], pm["fc_g1_b"]),
        "wg2": _wb(pm["fc_g2_W"], pm["fc_g2_b"]),
        "wr1": _wb(pm["red1_W"], pm["red1_b"]),
        "wr2": _wb(pm["red2_W"], pm["red2_b"]),
        "wr3": _wb(pm["red3_W"], pm["red3_b"]),
        "wgt": _wb(pm["gate_W"], pm["gate_b"]),
        "wf1": _wb(pm["fc1_W"], pm["fc1_b"]),
        "wf2": _wb(pm["fc2_W"], pm["fc2_b"]),
        "wo": _wb(pm["out_W"], pm["out_b"]),
    }

    x1 = np.ascontiguousarray(np.asarray(x1, dtype=np.float32))
    x2 = np.ascontiguousarray(np.asarray(x2, dtype=np.float32))
    cell = np.asarray(cell, dtype=np.float32)

    per_core = []
    for c in range(NCORES):
        sl = slice(c * NLOC, (c + 1) * NLOC)
        d = {
            "x1_full": x1,
            "x2_full": x2,
            "deg1_full": _deg_tiled(p1["deg"], N_NODES // 128),
            "deg2_full": _deg_tiled(p2["deg"], N_NODES // 128),
            "deg1_loc": _deg_tiled(p1["deg"][sl], NT),
            "deg2_loc": _deg_tiled(p2["deg"][sl], NT),
            "idx1": _idx_sbuf_layout(idx1[c]),
            "idx2": _idx_sbuf_layout(idx2[c]),
            "povf1": povf1[c],
            "povf2": povf2[c],
            "cell_loc": np.ascontiguousarray(cell[c * GLOC:(c + 1) * GLOC]),
            "ident": np.eye(128, dtype=BF16),
            "p32": np.repeat(np.eye(32, dtype=BF16), 4, axis=0),  # [128, 32]
        }
        for k, v in weights.items():
            d[k] = v
        per_core.append(d)

    meta = dict(m_ovf=m_ovf, gpt=17 + m_ovf)
    return meta, per_core


# ===========================================================================
# Numpy golden model of the DEVICE algorithm (bf16 rounding mirrored)
# ===========================================================================

def _bf(a):
    return a.astype(BF16)


def _golden_encoder(core_in, which, meta):
    """Mirror of the device program for one encoder on all cores -> dT [128, 512]."""
    m_ovf = meta["m_ovf"]
    gpt = meta["gpt"]
    xk = f"x{which}_full"
    x = core_in[0][xk]

    # replicated table 1 build
    degf = core_in[0][f"deg{which}_full"]  # [128, 200]
    dinv_full = 1.0 / np.sqrt(degf)        # device: reciprocal then sqrt
    # y1[node] = x[node] * dinv[node]; node = t*128+p
    dcol = dinv_full.T.reshape(-1)[:, None]  # [25600, 1]
    tab = np.zeros((TAB_ROWS, RE12), dtype=BF16)
    tab[:N_NODES, :F1] = _bf(x * dcol)

    d_all = []
    for lay, (fin, fout, re_in) in enumerate(
        [(F1, F1, RE12), (F1, F2, RE12), (F2, F3, RE3)]
    ):
        wname = f"w{lay + 1}"
        slabs = []
        for c in range(NCORES):
            ci = core_in[c]
            idx = ci[f"idx{which}"]  # [128, cols] i16
            ncols = idx.shape[1]
            flat = idx[:16].T.reshape(-1).astype(np.int64)  # undo layout
            flat = np.where(flat < 0, flat + 65536, flat)
            a = flat.reshape(NT, gpt, 128)
            degl = ci[f"deg{which}_loc"]       # [128, NT]
            dinv2_l = 1.0 / degl
            dinv_l = np.sqrt(dinv2_l)
            W = _bf(ci[wname])                  # [fin+1, fout]
            povf = ci[f"povf{which}"]
            slab = np.zeros((NLOC, fout), dtype=BF16)
            for t in range(NT):
                M = tab[a[t], :fin].astype(np.float32)  # [gpt, 128, fin]
                agg = M[16].copy()                       # self (identity)
                for qg in range(16):
                    q = qg // 4
                    part = M[qg].reshape(32, 4, fin).sum(axis=1)  # [32, fin]
                    agg[q * 32:(q + 1) * 32] += part
                for b in range(m_ovf):
                    P = povf[t, b].astype(np.float32)  # [128, 128]
                    agg += P.T @ M[17 + b]
                if lay < 2:
                    scale, col = dinv2_l[:, t], dinv_l[:, t]
                else:
                    scale, col = dinv_l[:, t], np.ones(128, np.float32)
                sp = np.concatenate(
                    [_bf(agg * scale[:, None]).astype(np.float32),
                     _bf(col)[:, None].astype(np.float32)], axis=1
                ).astype(BF16)
                h = sp.astype(np.float32) @ W.astype(np.float32)  # [128, fout]
                out = _bf(np.maximum(h, 0.0))
                slab[t * TILE:(t + 1) * TILE] = out
            slabs.append(slab)
        full = np.concatenate(slabs, axis=0)  # [25600, fout]
        if lay < 2:
            re_out = RE12 if lay == 0 else RE3
            tab = np.zeros((TAB_ROWS, re_out), dtype=BF16)
            tab[:N_NODES, :fout] = full
        else:
            h3 = full.astype(np.float32)  # relu'd conv3 out
    # pooling per core -> dT
    dT = []
    for c in range(NCORES):
        ci = core_in[c]
        hc = h3[c * NLOC:(c + 1) * NLOC]
        pooled = hc.reshape(GLOC, NPG, F3).max(axis=1)  # [64, 312]
        pooledT = _bf(pooled.T)                          # [312, 64]
        ones = np.ones((1, GLOC), dtype=BF16)
        g1 = np.concatenate([pooledT, ones]).astype(np.float32)
        fg1 = np.maximum(_bf(ci["wg1"]).astype(np.float32).T @ g1, 0.0)
        g2 = np.concatenate([_bf(fg1), ones]).astype(np.float32)
        d = _bf(ci["wg2"]).astype(np.float32).T @ g2     # [128, 64]
        dT.append(_bf(d))
    return dT  # list of [128, 64] bf16 per core


def golden_forward(inputs, meta, per_core):
    d1T = _golden_encoder(per_core, 1, meta)
    d2T = _golden_encoder(per_core, 2, meta)
    outs = []
    for c in range(NCORES):
        ci = per_core[c]
        ones = np.ones((1, GLOC), dtype=np.float32)
        cellc = ci["cell_loc"].astype(np.float32)  # [64, 954]
        ss = (cellc * cellc).sum(axis=1, keepdims=True)
        rn = 1.0 / np.maximum(np.sqrt(ss), 1e-12)
        cvn = _bf(cellc * rn)                      # [64, 954] bf16
        cvT = cvn.T.astype(np.float32)             # [954, 64]
        r1 = np.maximum(_bf(ci["wr1"]).astype(np.float32).T
                        @ np.concatenate([cvT, ones]), 0.0)
        r2 = np.maximum(_bf(ci["wr2"]).astype(np.float32).T
                        @ np.concatenate([_bf(r1).astype(np.float32), ones]), 0.0)
        cv = _bf(ci["wr3"]).astype(np.float32).T \
            @ np.concatenate([_bf(r2).astype(np.float32), ones])  # [128, 64]
        d1 = d1T[c].astype(np.float32)
        d2 = d2T[c].astype(np.float32)
        gate_in = np.concatenate([d1, d2, ones])
        gate = 1.0 / (1.0 + np.exp(-(_bf(ci["wgt"]).astype(np.float32).T @ gate_in)))
        xc = np.concatenate([d1, d2, _bf(gate).astype(np.float32),
                             _bf(cv).astype(np.float32), ones])
        f1 = np.maximum(_bf(ci["wf1"]).astype(np.float32).T @ xc, 0.0)
        f2 = np.maximum(_bf(ci["wf2"]).astype(np.float32).T
                        @ np.concatenate([_bf(f1).astype(np.float32), ones]), 0.0)
        o = _bf(ci["wo"]).astype(np.float32).T \
            @ np.concatenate([_bf(f2).astype(np.float32), ones])  # [2, 64]
        outs.append(o.T)
    return np.concatenate(outs, axis=0).astype(np.float32)  # [512, 2]


# ===========================================================================
# Bass device program
# ===========================================================================

CH12 = 4   # node tiles per dma_gather call, layers 1-2
CH3 = 2    # node tiles per dma_gather call, layer 3


WCHUNKS = {
    "w1": [F1 + 1], "w2": [F1 + 1], "w3": [128, F2 + 1 - 128],
    "wg1": [128, 128, F3 - 256, 1], "wg2": [128, 2 * FXD - 128, 1],
    "wr1": [128] * 7 + [FXT - 896, 1], "wr2": [128] * 4 + [1],
    "wr3": [128, 128, 1], "wgt": [128, 128, 1],
    "wf1": [128] * 4 + [1], "wf2": [128] * 4 + [1], "wo": [128, 1],
}


def build_bass(meta):
    import concourse.bass as bass
    import concourse.bacc as bacc
    import concourse.mybir as mybir
    import concourse.tile as tile

    F32 = mybir.dt.float32
    BF = mybir.dt.bfloat16
    I16 = mybir.dt.int16
    AF = mybir.ActivationFunctionType
    ALU = mybir.AluOpType

    m_ovf = meta["m_ovf"]
    gpt = meta["gpt"]
    IDXC = NT * gpt * 8  # idx cols per encoder

    nc = bacc.Bacc("TRN2", target_bir_lowering=False, debug=False,
                   num_devices=NCORES)

    def din(name, shape, dtype=F32):
        return nc.declare_dram_parameter(name, list(shape), dtype, isOutput=False)

    x_d = [din("x1_full", (N_NODES, FXD)), din("x2_full", (N_NODES, FXD))]
    degf_d = [din("deg1_full", (128, N_NODES // 128)),
              din("deg2_full", (128, N_NODES // 128))]
    degl_d = [din("deg1_loc", (128, NT)), din("deg2_loc", (128, NT))]
    idx_d = [din("idx1", (128, IDXC), I16), din("idx2", (128, IDXC), I16)]
    povf_d = [din("povf1", (NT, m_ovf, 128, 128), BF),
              din("povf2", (NT, m_ovf, 128, 128), BF)]
    cell_d = din("cell_loc", (GLOC, FXT))
    ident_d = din("ident", (128, 128), BF)
    p32_d = din("p32", (128, 32), BF)
    wshapes = {
        "w1": (F1 + 1, F1), "w2": (F1 + 1, F2), "w3": (F2 + 1, F3),
        "wg1": (F3 + 1, 2 * FXD), "wg2": (2 * FXD + 1, ODIM),
        "wr1": (FXT + 1, 512), "wr2": (513, 256), "wr3": (257, ODIM),
        "wgt": (2 * ODIM + 1, ODIM), "wf1": (513, 512), "wf2": (513, ODIM),
        "wo": (ODIM + 1, N_OUT),
    }
    for wn, (rows, _c) in wshapes.items():
        assert sum(WCHUNKS[wn]) == rows, wn
    w_d = {k: din(k, v) for k, v in wshapes.items()}
    out_d = nc.declare_dram_parameter("outT", [N_OUT, GLOC], F32, isOutput=True)

    RG = [list(range(NCORES))]

    with tile.TileContext(nc) as tc:
        with (
            tc.tile_pool(name="dram", bufs=1, space="DRAM") as dpool,
            tc.tile_pool(name="const", bufs=1) as cpool,
            tc.tile_pool(name="big", bufs=1) as bpool,
            tc.tile_pool(name="work", bufs=2) as wpool,
            tc.tile_pool(name="gath", bufs=2) as gpool,
            tc.tile_pool(name="stage", bufs=3) as spool,
            tc.tile_pool(name="psA", bufs=2, space="PSUM") as psA,
            tc.tile_pool(name="psB", bufs=2, space="PSUM") as psB,
            tc.tile_pool(name="psC", bufs=1, space="PSUM") as psC,
        ):
            # ---------------- DRAM scratch ----------------
            tab1 = [dpool.tile([TAB_ROWS, RE12], BF, tag=f"tab1_{k}", name=f"tab1_{k}") for k in range(2)]
            tab2 = [dpool.tile([TAB_ROWS, RE12], BF, tag=f"tab2_{k}", name=f"tab2_{k}") for k in range(2)]
            tab3 = [dpool.tile([TAB_ROWS, RE3], BF, tag=f"tab3_{k}", name=f"tab3_{k}") for k in range(2)]
            slab2 = [dpool.tile([NLOC, RE12], BF, tag=f"slab2_{k}", name=f"slab2_{k}") for k in range(2)]
            slab3 = [dpool.tile([NLOC, RE3], BF, tag=f"slab3_{k}", name=f"slab3_{k}") for k in range(2)]

            # ---------------- constants ----------------
            ident_sb = cpool.tile([128, 128], BF, tag="ident")
            nc.sync.dma_start(ident_sb[:], ident_d[:])
            p32_sb = cpool.tile([128, 32], BF, tag="p32")
            nc.sync.dma_start(p32_sb[:], p32_d[:])
            idx_sb = [cpool.tile([128, IDXC], I16, tag=f"idx{k}", name=f"idx{k}") for k in range(2)]
            for k in range(2):
                nc.sync.dma_start(idx_sb[k][:], idx_d[k][:])
            onesbf = cpool.tile([128, 1], BF, tag="onesbf")
            nc.vector.memset(onesbf[:], 1.0)
            ones64 = cpool.tile([128, GLOC], BF, tag="ones64")
            nc.vector.memset(ones64[:], 1.0)
            zrow = cpool.tile([1, RE3], BF, tag="zrow")
            nc.vector.memset(zrow[:], 0.0)
            for k in range(2):
                nc.sync.dma_start(tab1[k][ZROW:ZROW + 1, :], zrow[:, 0:RE12])
                nc.sync.dma_start(tab2[k][ZROW:ZROW + 1, :], zrow[:, 0:RE12])
                nc.sync.dma_start(tab3[k][ZROW:ZROW + 1, :], zrow[:, 0:RE3])

            # weights -> bf16 SBUF chunk tiles (chunk rows follow WCHUNKS)
            wsb = {}
            for wname, (rows, cols) in wshapes.items():
                chunks = []
                r0 = 0
                for nr in WCHUNKS[wname]:
                    r1 = r0 + nr
                    stg = spool.tile([128, cols], F32, tag="wstage")
                    nc.sync.dma_start(stg[0:nr, :], w_d[wname][r0:r1, :])
                    wt = cpool.tile([128, cols], BF, tag=f"{wname}_{r0}", name=f"w_{wname}_{r0}")
                    nc.vector.tensor_copy(wt[0:nr, :], stg[0:nr, :])
                    chunks.append((wt, nr))
                    r0 = r1
                wsb[wname] = chunks

            # dinv tiles per encoder
            dinvf = []   # [128, 200] f32 (full, for table-1 build)
            dinv2l = []  # [128, NT] f32
            dinvl = []   # [128, NT] f32
            dinvlbf = []  # [128, NT] bf16
            for k in range(2):
                dgf = spool.tile([128, N_NODES // 128], F32, tag="degf")
                nc.sync.dma_start(dgf[:], degf_d[k][:])
                df = cpool.tile([128, N_NODES // 128], F32, tag=f"dinvf{k}", name=f"dinvf{k}")
                nc.vector.reciprocal(df[:], dgf[:])       # 1/deg
                nc.scalar.sqrt(df[:], df[:])              # deg^-1/2
                dinvf.append(df)
                dgl = spool.tile([128, NT], F32, tag="degl")
                nc.sync.dma_start(dgl[:], degl_d[k][:])
                d2 = cpool.tile([128, NT], F32, tag=f"dinv2l{k}", name=f"dinv2l{k}")
                nc.vector.reciprocal(d2[:], dgl[:])       # 1/deg = dinv^2
                dinv2l.append(d2)
                d1 = cpool.tile([128, NT], F32, tag=f"dinvl{k}", name=f"dinvl{k}")
                nc.scalar.sqrt(d1[:], d2[:])
                dinvl.append(d1)
                db = cpool.tile([128, NT], BF, tag=f"dinvlbf{k}", name=f"dinvlbf{k}")
                nc.vector.tensor_copy(db[:], d1[:])
                dinvlbf.append(db)

            # registers for dma_gather num_idxs (allocate once per value;
            # per-call to_reg would exhaust the Pool register file)
            nidx_regs = {}

            def nidx_reg(v):
                if v not in nidx_regs:
                    nidx_regs[v] = nc.gpsimd.to_reg(v)
                return nidx_regs[v]

            # ---------------- encoders ----------------
            dT = []  # per encoder [128, GLOC] bf16
            for k in range(2):
                # table 1 = dinv * x (replicated build)
                NB = 4
                for tb in range(0, N_NODES // 128, NB):
                    xt = spool.tile([128, NB, FXD], F32, tag="xt")
                    src_ap = x_d[k][tb * 128:(tb + NB) * 128, :].rearrange(
                        "(t p) c -> p t c", p=128)
                    nc.sync.dma_start(xt[:], src_ap)
                    yt = spool.tile([128, NB, FXD], BF, tag="yt")
                    for i in range(NB):
                        eng = nc.vector if (tb // NB + i) % 2 == 0 else nc.scalar
                        if eng is nc.vector:
                            nc.vector.tensor_scalar_mul(
                                yt[:, i, :], xt[:, i, :],
                                dinvf[k][:, tb + i:tb + i + 1])
                        else:
                            nc.scalar.activation(
                                yt[:, i, :], xt[:, i, :], AF.Copy,
                                scale=dinvf[k][:, tb + i:tb + i + 1])
                    dst_ap = tab1[k][tb * 128:(tb + NB) * 128, 0:FXD].rearrange(
                        "(t p) c -> p t c", p=128)
                    nc.sync.dma_start(dst_ap, yt[:])

                # ---- layers ----
                for lay, (fin, fout, re_in, ch) in enumerate(
                    [(F1, F1, RE12, CH12), (F1, F2, RE12, CH12), (F2, F3, RE3, CH3)]
                ):
                    tab_in = [tab1, tab2, tab3][lay][k]
                    if lay == 2:
                        s3a = bpool.tile([128, NLOC], BF, tag="s3a")
                        s3b = bpool.tile([128, NLOC], BF, tag="s3b")
                    for t0 in range(0, NT, ch):
                        ntile = min(ch, NT - t0)
                        gcols = ntile * gpt * 8
                        c0 = t0 * gpt * 8
                        gbuf = gpool.tile([128, ntile * gpt, re_in], BF, tag="gbuf")
                        n_idx = ntile * gpt * 128
                        nc.gpsimd.dma_gather(
                            gbuf[:], tab_in[:, :],
                            idx_sb[k][:, c0:c0 + gcols],
                            n_idx, nidx_reg(n_idx), re_in, elem_step=re_in,
                            single_packet=False,
                        )
                        for tt in range(ntile):
                            t = t0 + tt
                            pv = spool.tile([128, m_ovf, 128], BF, tag="povf")
                            pv_src = povf_d[k][t, :, :, :].rearrange(
                                "b p n -> p b n")
                            nc.sync.dma_start(pv[:], pv_src)
                            agg = psA.tile([128, 512], F32, tag="agg")
                            base = tt * gpt
                            # each 32-row quarter is started by its first
                            # matmul; identity/overflow then accumulate over
                            # the whole tile and the last one stops the group
                            for qg in range(16):
                                q, g = qg // 4, qg % 4
                                nc.tensor.matmul(
                                    agg[q * 32:(q + 1) * 32, 0:fin], p32_sb[:, :],
                                    gbuf[:, base + qg, 0:fin],
                                    start=(g == 0), stop=False,
                                    skip_group_check=True,
                                    tile_position=(0, q * 32))
                            nc.tensor.matmul(
                                agg[:, 0:fin], ident_sb[:, :],
                                gbuf[:, base + 16, 0:fin],
                                start=False, stop=False,
                                skip_group_check=True)
                            for b in range(m_ovf):
                                nc.tensor.matmul(
                                    agg[:, 0:fin], pv[:, b, :],
                                    gbuf[:, base + 17 + b, 0:fin],
                                    start=False, stop=(b == m_ovf - 1),
                                    skip_group_check=True)
                            # s' = [scale*agg, col]
                            sp = wpool.tile([128, fin + 1], BF, tag="sp")
                            scale = dinv2l[k] if lay < 2 else dinvl[k]
                            nc.scalar.activation(
                                sp[:, 0:fin], agg[:, 0:fin], AF.Copy,
                                scale=scale[:, t:t + 1])
                            col = dinvlbf[k][:, t:t + 1] if lay < 2 \
                                else onesbf[:, 0:1]
                            nc.vector.tensor_copy(sp[:, fin:fin + 1], col)
                            # transpose s' -> [fin+1, 128]
                            spT = psB.tile([128, 1024], BF, tag="spT")
                            if fin + 1 <= 128:
                                nc.tensor.transpose(
                                    spT[0:fin + 1, 0:128], sp[:, :], ident_sb[:, :])
                            else:
                                nc.tensor.transpose(
                                    spT[:, 0:128], sp[:, 0:128], ident_sb[:, :])
                                spT2 = psC.tile([32, 1024], BF, tag="spT2")
                                nc.tensor.transpose(
                                    spT2[0:fin + 1 - 128, 0:128],
                                    sp[:, 128:fin + 1], ident_sb[:, :])
                            if lay < 2:
                                spT_sb = wpool.tile([128, 128], BF, tag="spT_sb")
                                nc.vector.tensor_copy(
                                    spT_sb[0:fin + 1, :], spT[0:fin + 1, 0:128])
                                h = psA.tile([128, 512], F32, tag="h")
                                (w_t, _), = wsb[f"w{lay + 1}"]
                                nc.tensor.matmul(
                                    h[:, 0:fout], spT_sb[0:fin + 1, :],
                                    w_t[0:fin + 1, 0:fout],
                                    start=True, stop=True)
                                slab_sb = wpool.tile([128, fout], BF, tag="slab")
                                nc.scalar.activation(
                                    slab_sb[:], h[:, 0:fout], AF.Relu)
                                slab_d = [slab2, slab3][lay][k]
                                nc.sync.dma_start(
                                    slab_d[t * 128:(t + 1) * 128, 0:fout],
                                    slab_sb[:])
                            else:
                                nc.vector.tensor_copy(
                                    s3a[:, t * 128:(t + 1) * 128], spT[:, 0:128])
                                nc.vector.tensor_copy(
                                    s3b[0:F2 + 1 - 128, t * 128:(t + 1) * 128],
                                    spT2[0:F2 + 1 - 128, 0:128])
                    if lay < 2:
                        slab_d = [slab2, slab3][lay][k]
                        tab_out = [tab2, tab3][lay][k]
                        nc.gpsimd.collective_compute(
                            "AllGather", ALU.bypass, replica_groups=RG,
                            ins=[slab_d[:, :]],
                            outs=[tab_out[0:N_NODES, :]],
                        )

                # ---- h3T = relu(W3'.T @ s3T), feature-major ----
                w3a, w3b = wsb["w3"][0][0], wsb["w3"][1][0]
                h3T = [bpool.tile([128, NLOC], BF, tag=f"h3T{i}", name=f"h3T{i}_{k}") for i in range(3)]
                fo_sz = [128, 128, F3 - 256]
                for fo in range(3):
                    for nb in range(0, NLOC, 512):
                        nbs = min(512, NLOC - nb)
                        h3 = psA.tile([128, 512], F32, tag="h")
                        nc.tensor.matmul(
                            h3[0:fo_sz[fo], 0:nbs],
                            w3a[0:128, fo * 128:fo * 128 + fo_sz[fo]],
                            s3a[:, nb:nb + nbs], start=True, stop=False,
                            skip_group_check=True)
                        nc.tensor.matmul(
                            h3[0:fo_sz[fo], 0:nbs],
                            w3b[0:F2 + 1 - 128, fo * 128:fo * 128 + fo_sz[fo]],
                            s3b[0:F2 + 1 - 128, nb:nb + nbs],
                            start=False, stop=True,
                            skip_group_check=True)
                        nc.scalar.activation(
                            h3T[fo][0:fo_sz[fo], nb:nb + nbs],
                            h3[0:fo_sz[fo], 0:nbs], AF.Relu)
                # ---- segment max pool: [fo, 64 graphs] ----
                pooledT = []
                for fo in range(3):
                    pt = wpool.tile([128, GLOC], BF, tag=f"pooledT{fo}", name=f"pooledT{fo}_{k}")
                    src3 = h3T[fo][0:fo_sz[fo], :].rearrange(
                        "p (g w) -> p g w", w=NPG)
                    nc.vector.reduce_max(
                        pt[0:fo_sz[fo], :], src3, axis=mybir.AxisListType.X)
                    pooledT.append((pt, fo_sz[fo]))
                pooledT.append((ones64, 1))

                # ---- fc_g1 (relu) + fc_g2 ----
                def dense_T(rhs_chunks, wname, act, tagp):
                    """rhs_chunks: list of (tile, rows). Returns list of
                    (tile [fo_sz, GLOC] bf16, fo_sz) per output chunk."""
                    chunks = wsb[wname]
                    assert len(chunks) == len(rhs_chunks), wname
                    fo_cols = wshapes[wname][1]
                    outs = []
                    for o0 in range(0, fo_cols, 128):
                        o1 = min(o0 + 128, fo_cols)
                        ps = psC.tile([128, 512], F32, tag="hps")
                        for ci, (wt, wr) in enumerate(chunks):
                            rhs_t, rr = rhs_chunks[ci]
                            assert rr == wr, (wname, ci, rr, wr)
                            nc.tensor.matmul(
                                ps[0:o1 - o0, 0:GLOC], wt[0:wr, o0:o1],
                                rhs_t[0:rr, :],
                                start=(ci == 0), stop=(ci == len(chunks) - 1))
                        ot = wpool.tile([128, GLOC], BF,
                                        tag=f"dout{tagp}_{o0}", name=f"dout{tagp}_{o0}")
                        osl = ot[0:o1 - o0, :]
                        if act == "relu":
                            nc.scalar.activation(osl, ps[0:o1 - o0, 0:GLOC], AF.Relu)
                        elif act == "sigmoid":
                            nc.scalar.activation(osl, ps[0:o1 - o0, 0:GLOC],
                                                 AF.Sigmoid)
                        else:
                            nc.vector.tensor_copy(osl, ps[0:o1 - o0, 0:GLOC])
                        outs.append((ot, o1 - o0))
                    return outs

                fg1 = dense_T(pooledT, "wg1", "relu", f"g1_{k}")
                fg1.append((ones64, 1))
                dk = dense_T(fg1, "wg2", "none", f"g2_{k}")
                dT.append(dk[0][0])

            # ---------------- head ----------------
            cell_sb = spool.tile([128, FXT], F32, tag="cell")
            nc.sync.dma_start(cell_sb[0:GLOC, :], cell_d[:])
            sq = wpool.tile([128, FXT], F32, tag="cellsq")
            nc.vector.tensor_mul(sq[0:GLOC, :], cell_sb[0:GLOC, :],
                                 cell_sb[0:GLOC, :])
            ss = wpool.tile([128, 1], F32, tag="cellss")
            nc.vector.reduce_sum(ss[0:GLOC, :], sq[0:GLOC, :],
                                 axis=mybir.AxisListType.X)
            nc.scalar.sqrt(ss[0:GLOC, :], ss[0:GLOC, :])
            nc.vector.tensor_scalar_max(ss[0:GLOC, :], ss[0:GLOC, :], 1e-12)
            rn = wpool.tile([128, 1], F32, tag="cellrn")
            nc.vector.reciprocal(rn[0:GLOC, :], ss[0:GLOC, :])
            cvn = wpool.tile([128, FXT], BF, tag="cvn")
            nc.vector.tensor_scalar_mul(cvn[0:GLOC, :], cell_sb[0:GLOC, :],
                                        rn[0:GLOC, :])
            # transpose cvn -> cellT chunks
            cellT = []
            r0 = 0
            while r0 < FXT:
                r1 = min(r0 + 128, FXT)
                cps = psC.tile([128, 1024], BF, tag="hps")
                nc.tensor.transpose(
                    cps[0:r1 - r0, 0:GLOC], cvn[0:GLOC, r0:r1],
                    ident_sb[0:GLOC, 0:GLOC])
                ct = wpool.tile([128, GLOC], BF, tag=f"cellT{r0}", name=f"cellT{r0}")
                nc.vector.tensor_copy(ct[0:r1 - r0, :], cps[0:r1 - r0, 0:GLOC])
                cellT.append((ct, r1 - r0))
                r0 = r1
            cellT.append((ones64, 1))

            def with_ones(chunks):
                return list(chunks) + [(ones64, 1)]

            r1o = dense_T(cellT, "wr1", "relu", "r1")
            r2o = dense_T(with_ones(r1o), "wr2", "relu", "r2")
            cvo = dense_T(with_ones(r2o), "wr3", "none", "r3")
            gate = dense_T([(dT[0], 128), (dT[1], 128), (ones64, 1)],
                           "wgt", "sigmoid", "gt")
            xc = [(dT[0], 128), (dT[1], 128), (gate[0][0], 128),
                  (cvo[0][0], 128), (ones64, 1)]
            f1o = dense_T(xc, "wf1", "relu", "f1")
            f2o = dense_T(with_ones(f1o), "wf2", "relu", "f2")
            oo = dense_T(with_ones(f2o), "wo", "none", "oo")
            out_sb = wpool.tile([128, GLOC], F32, tag="outsb")
            nc.vector.tensor_copy(out_sb[0:N_OUT, :], oo[0][0][0:N_OUT, :])
            nc.sync.dma_start(out_d[:, :], out_sb[0:N_OUT, :])

    nc.compile()
    return nc


# dense_T writes bf16 outputs; the final "oo" needs f32 — handled by copying
# the bf16 tile to f32 out_sb above (acceptable: last layer output rounding).


# ===========================================================================
# Entry point
# ===========================================================================

_CACHE = {}


INPUT_NAMES = ("x1_full", "x2_full", "deg1_full", "deg2_full", "deg1_loc",
               "deg2_loc", "idx1", "idx2", "povf1", "povf2", "cell_loc",
               "ident", "p32", "w1", "w2", "w3", "wg1", "wg2", "wr1", "wr2",
               "wr3", "wgt", "wf1", "wf2", "wo")


def run_device(meta, per_core, **kwargs):
    """Compile (cached) + run on the 8 NeuronCores. Returns (out, results)."""
    from concourse.bass_utils import run_bass_kernel_spmd

    key = meta["m_ovf"]
    if key not in _CACHE:
        _CACHE[key] = build_bass(meta)
    nc = _CACHE[key]

    in_maps = [{n: per_core[c][n] for n in INPUT_NAMES} for c in range(NCORES)]
    res = run_bass_kernel_spmd(nc, in_maps, list(range(NCORES)), **kwargs)
    outs = [np.asarray(res.results[c]["outT"]) for c in range(NCORES)]
    out = np.concatenate([o.T for o in outs], axis=0).astype(np.float32)
    return out, res


def kernel(x1, edge_index1, batch1, x2, edge_index2, batch2, cell, params):
    meta, per_core = prep_all(x1, edge_index1, x2, edge_index2, cell, params)
    out, _ = run_device(meta, per_core)
    return out


# revision 37
# speedup vs baseline: 1.4790x; 1.4790x over previous
"""Trainium2 Bass kernel for nn_GCGNet (2x GCN drug encoder + dense fusion head).

Strategy (8 NeuronCores, SPMD):
  - Shard target nodes / graphs: core c owns nodes [c*3200, (c+1)*3200) and
    graphs [c*64, (c+1)*64).
  - GCNConv reformulation: with deg = indegree+1, dinv = deg^-1/2,
    y = dinv * x (row scale), each layer's pre-activation out rows are
        out = dinv * (A @ y + y) @ W + b
    so the gather tables hold y (bf16, 256B-aligned rows in DRAM) and the
    edge coefficient vanishes: the scatter matrix is pure 0/1.
  - Aggregation A @ y: fixed-degree main slots (16 per node) gathered with
    dma_gather, reduced on the TensorEngine with a constant one-hot
    P16 [128,16->8] (slot partition p feeds node p//16), plus a self group
    (identity) and a few shipped one-hot overflow blocks for deg > 16.
  - Bias folding: s' = [scale*s, col] @ [W; b] with (scale, col) =
    (dinv^2, dinv) for layers 1-2 (fuses the next table's y scaling) and
    (dinv, 1) for layer 3.
  - Layers 1-2 write node-major table slabs -> AllGather across the 8 cores.
    Layer 3 emits feature-major h3T for segment-max pooling; the whole dense
    fusion head is graph-local per core, output [2, 64] per core.
"""

import numpy as np
import ml_dtypes

BF16 = ml_dtypes.bfloat16

# ---- problem constants (hardcoded per the task contract) -------------------
N_NODES = 25600
N_EDGES = 409600
N_GRAPHS = 512
FXD = 78
FXT = 954
ODIM = 128
N_OUT = 2

NCORES = 8
NLOC = N_NODES // NCORES          # 3200 nodes per core
GLOC = N_GRAPHS // NCORES         # 64 graphs per core
NPG = N_NODES // N_GRAPHS         # 50 nodes per graph
TILE = 128
NT = NLOC // TILE                 # 25 node tiles per core

D_MAIN = 16                       # fixed-degree main slots per node
ZROW = N_NODES                    # index of the all-zero table row
TAB_ROWS = N_NODES + TILE         # 25728
RE12 = 128                        # table row elems (bf16) for layers 1,2 (F=78)
RE3 = 256                         # table row elems (bf16) for layer 3 (F=156)

F1, F2, F3 = FXD, 2 * FXD, 4 * FXD   # 78, 156, 312


# ===========================================================================
# Host-side prep: sharding + index structures (pure integer work)
# ===========================================================================

def _prep_encoder(edge_index):
    """Build per-core gather/scatter index structures for one edge set.

    Returns dict with:
      deg        [N_NODES] f32    (indegree + 1)
      slots      [NCORES, NLOC, D_MAIN] int32   (src ids, ZROW = pad)
      ovf_src    list-of-lists per (core, tile): np arrays of src ids
      ovf_tloc   same shape: target offset within the 128-node tile
    """
    src = edge_index[0].astype(np.int64)
    tgt = edge_index[1].astype(np.int64)
    counts = np.bincount(tgt, minlength=N_NODES)
    deg = counts.astype(np.float32) + 1.0

    order = np.argsort(tgt, kind="stable")
    ssrc = src[order]
    stgt = tgt[order]
    rowptr = np.zeros(N_NODES + 1, np.int64)
    rowptr[1:] = np.cumsum(counts)
    pos = np.arange(N_EDGES, dtype=np.int64) - rowptr[stgt]

    # slot 0 is the self row (edge n->n, coefficient 1 in A+I); slots 1..15
    # take the first 15 in-edges, the rest overflow
    main_mask = pos < D_MAIN - 1
    slots = np.full((N_NODES, D_MAIN), ZROW, np.int32)
    slots[:, 0] = np.arange(N_NODES, dtype=np.int32)
    slots[stgt[main_mask], pos[main_mask] + 1] = ssrc[main_mask]
    slots = slots.reshape(NCORES, NLOC, D_MAIN)

    o_src = ssrc[~main_mask]
    o_tgt = stgt[~main_mask]
    ovf_src = [[None] * NT for _ in range(NCORES)]
    ovf_tloc = [[None] * NT for _ in range(NCORES)]
    tile_id = o_tgt // TILE
    for c in range(NCORES):
        for t in range(NT):
            m = tile_id == c * NT + t
            ovf_src[c][t] = o_src[m]
            ovf_tloc[c][t] = (o_tgt[m] - (c * NT + t) * TILE).astype(np.int64)
    return dict(deg=deg, slots=slots, ovf_src=ovf_src, ovf_tloc=ovf_tloc)


def _max_ovf_blocks(preps):
    m = 1
    for pr in preps:
        for c in range(NCORES):
            for t in range(NT):
                m = max(m, -(-len(pr["ovf_src"][c][t]) // TILE))
    return m


def _build_core_arrays(pr, m_ovf):
    """Per-core flat gather-index arrays + overflow one-hot blocks.

    idx   [NCORES, NT*(17+m_ovf)*128] int32  slot order:
          tile t, group g (0..15 main, 16 self, 17.. ovf), partition p
    povf  [NCORES, NT, m_ovf, 128, 128] bf16 one-hot (slot p, local node)
    """
    gpt = 16 + m_ovf
    idx = np.full((NCORES, NT * gpt * 128), ZROW, np.int32)
    povf = np.zeros((NCORES, NT, m_ovf, TILE, TILE), dtype=BF16)
    slots = pr["slots"]
    for c in range(NCORES):
        a = idx[c].reshape(NT, gpt, 128)
        for t in range(NT):
            base = t * TILE
            # main groups: group index qg = q*4+g covers nodes
            # base + q*32 + p//4 with edge slot j = g*4 + p%4
            qg_idx = np.arange(16)
            p_idx = np.arange(128)
            q = qg_idx[:, None] // 4
            g = qg_idx[:, None] % 4
            nodes = base + q * 32 + p_idx[None, :] // 4        # [16,128]
            js = g * 4 + p_idx[None, :] % 4                    # [16,128]
            a[t, :16, :] = slots[c][nodes, js]
            # overflow groups
            osrc = pr["ovf_src"][c][t]
            otl = pr["ovf_tloc"][c][t]
            for b in range(m_ovf):
                lo, hi = b * TILE, min((b + 1) * TILE, len(osrc))
                if lo >= len(osrc):
                    break
                n = hi - lo
                a[t, 16 + b, :n] = osrc[lo:hi]
                povf[c, t, b, np.arange(n), otl[lo:hi]] = 1.0
    return idx, povf


def _idx_sbuf_layout(idx_flat):
    """int32 flat slot list -> int16 [128, len/16] SBUF image.

    dma_gather reads index i from partition i%16, column i//16; replicate
    across the 8 groups of 16 partitions for the 8 Q7 cores.
    """
    n = idx_flat.shape[0]
    assert n % 16 == 0
    a = idx_flat.reshape(n // 16, 16).T.astype(np.int16)  # [16, n/16]
    return np.tile(a, (8, 1))                              # [128, n/16]


def _wb(W, b):
    """Stack [W; b] -> [in+1, out] f32."""
    return np.concatenate([W, b[None, :]], axis=0).astype(np.float32)


def _deg_tiled(deg, ntiles):
    """deg [ntiles*128] -> [128, ntiles] (partition p, col t = deg[t*128+p])."""
    return deg.reshape(ntiles, 128).T.copy()


def prep_all(x1, edge_index1, x2, edge_index2, cell, params):
    """All host-side prep. Returns (meta, per_core_inputs list)."""
    p1 = _prep_encoder(np.asarray(edge_index1))
    p2 = _prep_encoder(np.asarray(edge_index2))
    m_ovf = _max_ovf_blocks([p1, p2])
    idx1, povf1 = _build_core_arrays(p1, m_ovf)
    idx2, povf2 = _build_core_arrays(p2, m_ovf)

    pm = params
    weights = {
        "w1": _wb(pm["conv1_W"], pm["conv1_b"]),
        "w2": _wb(pm["conv2_W"], pm["conv2_b"]),
        "w3": _wb(pm["conv3_W"], pm["conv3_b"]),
        "wg1": _wb(pm["fc_g1_W"], pm["fc_g1_b"]),
        "wg2": _wb(pm["fc_g2_W"], pm["fc_g2_b"]),
        "wr1": _wb(pm["red1_W"], pm["red1_b"]),
        "wr2": _wb(pm["red2_W"], pm["red2_b"]),
        "wr3": _wb(pm["red3_W"], pm["red3_b"]),
        "wgt": _wb(pm["gate_W"], pm["gate_b"]),
        "wf1": _wb(pm["fc1_W"], pm["fc1_b"]),
        "wf2": _wb(pm["fc2_W"], pm["fc2_b"]),
        "wo": _wb(pm["out_W"], pm["out_b"]),
    }

    x1 = np.ascontiguousarray(np.asarray(x1, dtype=np.float32))
    x2 = np.ascontiguousarray(np.asarray(x2, dtype=np.float32))
    cell = np.asarray(cell, dtype=np.float32)

    per_core = []
    for c in range(NCORES):
        sl = slice(c * NLOC, (c + 1) * NLOC)
        d = {
            "x1_full": x1,
            "x2_full": x2,
            "deg1_full": _deg_tiled(p1["deg"], N_NODES // 128),
            "deg2_full": _deg_tiled(p2["deg"], N_NODES // 128),
            "deg1_loc": _deg_tiled(p1["deg"][sl], NT),
            "deg2_loc": _deg_tiled(p2["deg"][sl], NT),
            "idx1": _idx_sbuf_layout(idx1[c]),
            "idx2": _idx_sbuf_layout(idx2[c]),
            "povf1": povf1[c],
            "povf2": povf2[c],
            "cell_loc": np.ascontiguousarray(cell[c * GLOC:(c + 1) * GLOC]),
            "ident": np.eye(128, dtype=BF16),
            "p32": np.repeat(np.eye(32, dtype=BF16), 4, axis=0),  # [128, 32]
        }
        for k, v in weights.items():
            d[k] = v
        per_core.append(d)

    meta = dict(m_ovf=m_ovf, gpt=16 + m_ovf)
    return meta, per_core


# ===========================================================================
# Numpy golden model of the DEVICE algorithm (bf16 rounding mirrored)
# ===========================================================================

def _bf(a):
    return a.astype(BF16)


def _golden_encoder(core_in, which, meta):
    """Mirror of the device program for one encoder on all cores -> dT [128, 512]."""
    m_ovf = meta["m_ovf"]
    gpt = meta["gpt"]
    xk = f"x{which}_full"
    x = core_in[0][xk]

    # replicated table 1 build
    degf = core_in[0][f"deg{which}_full"]  # [128, 200]
    dinv_full = 1.0 / np.sqrt(degf)        # device: reciprocal then sqrt
    # y1[node] = x[node] * dinv[node]; node = t*128+p
    dcol = dinv_full.T.reshape(-1)[:, None]  # [25600, 1]
    tab = np.zeros((TAB_ROWS, RE12), dtype=BF16)
    tab[:N_NODES, :F1] = _bf(x * dcol)

    d_all = []
    for lay, (fin, fout, re_in) in enumerate(
        [(F1, F1, RE12), (F1, F2, RE12), (F2, F3, RE3)]
    ):
        wname = f"w{lay + 1}"
        slabs = []
        for c in range(NCORES):
            ci = core_in[c]
            idx = ci[f"idx{which}"]  # [128, cols] i16
            ncols = idx.shape[1]
            flat = idx[:16].T.reshape(-1).astype(np.int64)  # undo layout
            flat = np.where(flat < 0, flat + 65536, flat)
            a = flat.reshape(NT, gpt, 128)
            degl = ci[f"deg{which}_loc"]       # [128, NT]
            dinv2_l = 1.0 / degl
            dinv_l = np.sqrt(dinv2_l)
            W = _bf(ci[wname])                  # [fin+1, fout]
            povf = ci[f"povf{which}"]
            slab = np.zeros((NLOC, fout), dtype=BF16)
            for t in range(NT):
                M = tab[a[t], :fin].astype(np.float32)  # [gpt, 128, fin]
                agg = np.zeros((128, fin), np.float32)
                for qg in range(16):
                    q = qg // 4
                    part = M[qg].reshape(32, 4, fin).sum(axis=1)  # [32, fin]
                    agg[q * 32:(q + 1) * 32] += part
                for b in range(m_ovf):
                    P = povf[t, b].astype(np.float32)  # [128, 128]
                    agg += P.T @ M[16 + b]
                if lay < 2:
                    scale, col = dinv2_l[:, t], dinv_l[:, t]
                else:
                    scale, col = dinv_l[:, t], np.ones(128, np.float32)
                sp = np.concatenate(
                    [_bf(agg * scale[:, None]).astype(np.float32),
                     _bf(col)[:, None].astype(np.float32)], axis=1
                ).astype(BF16)
                h = sp.astype(np.float32) @ W.astype(np.float32)  # [128, fout]
                out = _bf(np.maximum(h, 0.0))
                slab[t * TILE:(t + 1) * TILE] = out
            slabs.append(slab)
        full = np.concatenate(slabs, axis=0)  # [25600, fout]
        if lay < 2:
            re_out = RE12 if lay == 0 else RE3
            tab = np.zeros((TAB_ROWS, re_out), dtype=BF16)
            tab[:N_NODES, :fout] = full
        else:
            h3 = full.astype(np.float32)  # relu'd conv3 out
    # pooling per core -> dT
    dT = []
    for c in range(NCORES):
        ci = core_in[c]
        hc = h3[c * NLOC:(c + 1) * NLOC]
        pooled = hc.reshape(GLOC, NPG, F3).max(axis=1)  # [64, 312]
        pooledT = _bf(pooled.T)                          # [312, 64]
        ones = np.ones((1, GLOC), dtype=BF16)
        g1 = np.concatenate([pooledT, ones]).astype(np.float32)
        fg1 = np.maximum(_bf(ci["wg1"]).astype(np.float32).T @ g1, 0.0)
        g2 = np.concatenate([_bf(fg1), ones]).astype(np.float32)
        d = _bf(ci["wg2"]).astype(np.float32).T @ g2     # [128, 64]
        dT.append(_bf(d))
    return dT  # list of [128, 64] bf16 per core


def golden_forward(inputs, meta, per_core):
    d1T = _golden_encoder(per_core, 1, meta)
    d2T = _golden_encoder(per_core, 2, meta)
    outs = []
    for c in range(NCORES):
        ci = per_core[c]
        ones = np.ones((1, GLOC), dtype=np.float32)
        cellc = ci["cell_loc"].astype(np.float32)  # [64, 954]
        ss = (cellc * cellc).sum(axis=1, keepdims=True)
        rn = 1.0 / np.maximum(np.sqrt(ss), 1e-12)
        cvn = _bf(cellc * rn)                      # [64, 954] bf16
        cvT = cvn.T.astype(np.float32)             # [954, 64]
        r1 = np.maximum(_bf(ci["wr1"]).astype(np.float32).T
                        @ np.concatenate([cvT, ones]), 0.0)
        r2 = np.maximum(_bf(ci["wr2"]).astype(np.float32).T
                        @ np.concatenate([_bf(r1).astype(np.float32), ones]), 0.0)
        cv = _bf(ci["wr3"]).astype(np.float32).T \
            @ np.concatenate([_bf(r2).astype(np.float32), ones])  # [128, 64]
        d1 = d1T[c].astype(np.float32)
        d2 = d2T[c].astype(np.float32)
        gate_in = np.concatenate([d1, d2, ones])
        gate = 1.0 / (1.0 + np.exp(-(_bf(ci["wgt"]).astype(np.float32).T @ gate_in)))
        xc = np.concatenate([d1, d2, _bf(gate).astype(np.float32),
                             _bf(cv).astype(np.float32), ones])
        f1 = np.maximum(_bf(ci["wf1"]).astype(np.float32).T @ xc, 0.0)
        f2 = np.maximum(_bf(ci["wf2"]).astype(np.float32).T
                        @ np.concatenate([_bf(f1).astype(np.float32), ones]), 0.0)
        o = _bf(ci["wo"]).astype(np.float32).T \
            @ np.concatenate([_bf(f2).astype(np.float32), ones])  # [2, 64]
        outs.append(o.T)
    return np.concatenate(outs, axis=0).astype(np.float32)  # [512, 2]


# ===========================================================================
# Bass device program
# ===========================================================================

CH12 = 4   # node tiles per dma_gather call, layers 1-2
CH3 = 2    # node tiles per dma_gather call, layer 3


WCHUNKS = {
    "w1": [F1 + 1], "w2": [F1 + 1], "w3": [128, F2 + 1 - 128],
    "wg1": [128, 128, F3 - 256, 1], "wg2": [128, 2 * FXD - 128, 1],
    "wr1": [128] * 7 + [FXT - 896, 1], "wr2": [128] * 4 + [1],
    "wr3": [128, 128, 1], "wgt": [128, 128, 1],
    "wf1": [128] * 4 + [1], "wf2": [128] * 4 + [1], "wo": [128, 1],
}


def build_bass(meta):
    import concourse.bass as bass
    import concourse.bacc as bacc
    import concourse.mybir as mybir
    import concourse.tile as tile

    F32 = mybir.dt.float32
    BF = mybir.dt.bfloat16
    I16 = mybir.dt.int16
    AF = mybir.ActivationFunctionType
    ALU = mybir.AluOpType

    m_ovf = meta["m_ovf"]
    gpt = meta["gpt"]
    IDXC = NT * gpt * 8  # idx cols per encoder

    nc = bacc.Bacc("TRN2", target_bir_lowering=False, debug=False,
                   num_devices=NCORES)

    def din(name, shape, dtype=F32):
        return nc.declare_dram_parameter(name, list(shape), dtype, isOutput=False)

    x_d = [din("x1_full", (N_NODES, FXD)), din("x2_full", (N_NODES, FXD))]
    degf_d = [din("deg1_full", (128, N_NODES // 128)),
              din("deg2_full", (128, N_NODES // 128))]
    degl_d = [din("deg1_loc", (128, NT)), din("deg2_loc", (128, NT))]
    idx_d = [din("idx1", (128, IDXC), I16), din("idx2", (128, IDXC), I16)]
    povf_d = [din("povf1", (NT, m_ovf, 128, 128), BF),
              din("povf2", (NT, m_ovf, 128, 128), BF)]
    cell_d = din("cell_loc", (GLOC, FXT))
    ident_d = din("ident", (128, 128), BF)
    p32_d = din("p32", (128, 32), BF)
    wshapes = {
        "w1": (F1 + 1, F1), "w2": (F1 + 1, F2), "w3": (F2 + 1, F3),
        "wg1": (F3 + 1, 2 * FXD), "wg2": (2 * FXD + 1, ODIM),
        "wr1": (FXT + 1, 512), "wr2": (513, 256), "wr3": (257, ODIM),
        "wgt": (2 * ODIM + 1, ODIM), "wf1": (513, 512), "wf2": (513, ODIM),
        "wo": (ODIM + 1, N_OUT),
    }
    for wn, (rows, _c) in wshapes.items():
        assert sum(WCHUNKS[wn]) == rows, wn
    w_d = {k: din(k, v) for k, v in wshapes.items()}
    out_d = nc.declare_dram_parameter("outT", [N_OUT, GLOC], F32, isOutput=True)

    RG = [list(range(NCORES))]

    with tile.TileContext(nc) as tc:
        with (
            tc.tile_pool(name="dram", bufs=1, space="DRAM") as dpool,
            tc.tile_pool(name="const", bufs=1) as cpool,
            tc.tile_pool(name="big", bufs=1) as bpool,
            tc.tile_pool(name="work", bufs=2) as wpool,
            tc.tile_pool(name="gath", bufs=3) as gpool,
            tc.tile_pool(name="stage", bufs=3) as spool,
            tc.tile_pool(name="psA", bufs=2, space="PSUM") as psA,
            tc.tile_pool(name="psB", bufs=2, space="PSUM") as psB,
            tc.tile_pool(name="psC", bufs=1, space="PSUM") as psC,
        ):
            # ---------------- DRAM scratch ----------------
            tab1 = [dpool.tile([TAB_ROWS, RE12], BF, tag=f"tab1_{k}", name=f"tab1_{k}") for k in range(2)]
            tab2 = [dpool.tile([TAB_ROWS, RE12], BF, tag=f"tab2_{k}", name=f"tab2_{k}") for k in range(2)]
            tab3 = [dpool.tile([TAB_ROWS, RE3], BF, tag=f"tab3_{k}", name=f"tab3_{k}") for k in range(2)]
            slab2 = [dpool.tile([NLOC, RE12], BF, tag=f"slab2_{k}", name=f"slab2_{k}") for k in range(2)]
            slab3 = [dpool.tile([NLOC, RE3], BF, tag=f"slab3_{k}", name=f"slab3_{k}") for k in range(2)]

            # ---------------- constants ----------------
            ident_sb = cpool.tile([128, 128], BF, tag="ident")
            nc.sync.dma_start(ident_sb[:], ident_d[:])
            p32_sb = cpool.tile([128, 32], BF, tag="p32")
            nc.sync.dma_start(p32_sb[:], p32_d[:])
            idx_sb = [cpool.tile([128, IDXC], I16, tag=f"idx{k}", name=f"idx{k}") for k in range(2)]
            for k in range(2):
                nc.sync.dma_start(idx_sb[k][:], idx_d[k][:])
            onesbf = cpool.tile([128, 1], BF, tag="onesbf")
            nc.vector.memset(onesbf[:], 1.0)
            ones64 = cpool.tile([128, GLOC], BF, tag="ones64")
            nc.vector.memset(ones64[:], 1.0)
            zrow = cpool.tile([1, RE3], BF, tag="zrow")
            nc.vector.memset(zrow[:], 0.0)
            for k in range(2):
                nc.sync.dma_start(tab1[k][ZROW:ZROW + 1, :], zrow[:, 0:RE12])
                nc.sync.dma_start(tab2[k][ZROW:ZROW + 1, :], zrow[:, 0:RE12])
                nc.sync.dma_start(tab3[k][ZROW:ZROW + 1, :], zrow[:, 0:RE3])

            # weights -> bf16 SBUF chunk tiles (chunk rows follow WCHUNKS)
            wsb = {}
            for wname, (rows, cols) in wshapes.items():
                chunks = []
                r0 = 0
                for nr in WCHUNKS[wname]:
                    r1 = r0 + nr
                    stg = spool.tile([128, cols], F32, tag="wstage")
                    nc.sync.dma_start(stg[0:nr, :], w_d[wname][r0:r1, :])
                    wt = cpool.tile([128, cols], BF, tag=f"{wname}_{r0}", name=f"w_{wname}_{r0}")
                    nc.vector.tensor_copy(wt[0:nr, :], stg[0:nr, :])
                    chunks.append((wt, nr))
                    r0 = r1
                wsb[wname] = chunks

            # dinv tiles per encoder
            dinvf = []   # [128, 200] f32 (full, for table-1 build)
            dinv2l = []  # [128, NT] f32
            dinvl = []   # [128, NT] f32
            dinvlbf = []  # [128, NT] bf16
            for k in range(2):
                dgf = spool.tile([128, N_NODES // 128], F32, tag="degf")
                nc.sync.dma_start(dgf[:], degf_d[k][:])
                df = cpool.tile([128, N_NODES // 128], F32, tag=f"dinvf{k}", name=f"dinvf{k}")
                nc.vector.reciprocal(df[:], dgf[:])       # 1/deg
                nc.scalar.sqrt(df[:], df[:])              # deg^-1/2
                dinvf.append(df)
                dgl = spool.tile([128, NT], F32, tag="degl")
                nc.sync.dma_start(dgl[:], degl_d[k][:])
                d2 = cpool.tile([128, NT], F32, tag=f"dinv2l{k}", name=f"dinv2l{k}")
                nc.vector.reciprocal(d2[:], dgl[:])       # 1/deg = dinv^2
                dinv2l.append(d2)
                d1 = cpool.tile([128, NT], F32, tag=f"dinvl{k}", name=f"dinvl{k}")
                nc.scalar.sqrt(d1[:], d2[:])
                dinvl.append(d1)
                db = cpool.tile([128, NT], BF, tag=f"dinvlbf{k}", name=f"dinvlbf{k}")
                nc.vector.tensor_copy(db[:], d1[:])
                dinvlbf.append(db)

            # registers for dma_gather num_idxs (allocate once per value;
            # per-call to_reg would exhaust the Pool register file)
            nidx_regs = {}

            def nidx_reg(v):
                if v not in nidx_regs:
                    nidx_regs[v] = nc.gpsimd.to_reg(v)
                return nidx_regs[v]

            def dense_T(rhs_chunks, wname, act, tagp):
                """rhs_chunks: list of (tile, rows). Returns list of
                (tile [fo_sz, GLOC] bf16, fo_sz) per output chunk."""
                chunks = wsb[wname]
                assert len(chunks) == len(rhs_chunks), wname
                fo_cols = wshapes[wname][1]
                outs = []
                for o0 in range(0, fo_cols, 128):
                    o1 = min(o0 + 128, fo_cols)
                    ps = psC.tile([128, 512], F32, tag="hps")
                    for ci, (wt, wr) in enumerate(chunks):
                        rhs_t, rr = rhs_chunks[ci]
                        assert rr == wr, (wname, ci, rr, wr)
                        nc.tensor.matmul(
                            ps[0:o1 - o0, 0:GLOC], wt[0:wr, o0:o1],
                            rhs_t[0:rr, :],
                            start=(ci == 0), stop=(ci == len(chunks) - 1))
                    ot = wpool.tile([128, GLOC], BF,
                                    tag=f"dout{tagp}_{o0}", name=f"dout{tagp}_{o0}")
                    osl = ot[0:o1 - o0, :]
                    if act == "relu":
                        nc.scalar.activation(osl, ps[0:o1 - o0, 0:GLOC], AF.Relu)
                    elif act == "sigmoid":
                        nc.scalar.activation(osl, ps[0:o1 - o0, 0:GLOC],
                                             AF.Sigmoid)
                    else:
                        nc.vector.tensor_copy(osl, ps[0:o1 - o0, 0:GLOC])
                    outs.append((ot, o1 - o0))
                return outs

            # ---------------- encoders ----------------
            dT = []  # per encoder [128, GLOC] bf16
            s3ab = {}
            for k in range(2):
                # table 1 = dinv * x (replicated build)
                NB = 4
                for tb in range(0, N_NODES // 128, NB):
                    xt = spool.tile([128, NB, FXD], F32, tag="xt")
                    src_ap = x_d[k][tb * 128:(tb + NB) * 128, :].rearrange(
                        "(t p) c -> p t c", p=128)
                    nc.sync.dma_start(xt[:], src_ap)
                    yt = spool.tile([128, NB, FXD], BF, tag="yt")
                    for i in range(NB):
                        eng = nc.vector if (tb // NB + i) % 2 == 0 else nc.scalar
                        if eng is nc.vector:
                            nc.vector.tensor_scalar_mul(
                                yt[:, i, :], xt[:, i, :],
                                dinvf[k][:, tb + i:tb + i + 1])
                        else:
                            nc.scalar.activation(
                                yt[:, i, :], xt[:, i, :], AF.Copy,
                                scale=dinvf[k][:, tb + i:tb + i + 1])
                    dst_ap = tab1[k][tb * 128:(tb + NB) * 128, 0:FXD].rearrange(
                        "(t p) c -> p t c", p=128)
                    nc.sync.dma_start(dst_ap, yt[:])

            # ---- layers: lay-major so the two encoders interleave and
            # encoder B's gathers fill encoder A's AllGather stalls ----
            for lay, (fin, fout, re_in, ch) in enumerate(
                [(F1, F1, RE12, CH12), (F1, F2, RE12, CH12), (F2, F3, RE3, CH3)]
            ):
                for k in range(2):
                    tab_in = [tab1, tab2, tab3][lay][k]
                    if lay == 2:
                        s3a = bpool.tile([128, NLOC], BF, tag=f"s3a_{k}",
                                         name=f"s3a_{k}")
                        s3b = bpool.tile([128, NLOC], BF, tag=f"s3b_{k}",
                                         name=f"s3b_{k}")
                        s3ab[k] = (s3a, s3b)
                    for t0 in range(0, NT, ch):
                        ntile = min(ch, NT - t0)
                        gcols = ntile * gpt * 8
                        c0 = t0 * gpt * 8
                        gbuf = gpool.tile([128, ntile * gpt, re_in], BF, tag="gbuf")
                        n_idx = ntile * gpt * 128
                        # 512-idx sub-calls: 32 descriptors per SDMA queue
                        # keeps single_packet mode legal (64/queue limit),
                        # which is ~an order of magnitude cheaper per row
                        # on the Q7 descriptor generator than per-row packets
                        SUB = 1024
                        for s0 in range(0, n_idx, SUB):
                            ns = min(SUB, n_idx - s0)
                            g0, g1 = s0 // 128, (s0 + ns) // 128
                            nc.gpsimd.dma_gather(
                                gbuf[:, g0:g1, :], tab_in[:, :],
                                idx_sb[k][:, c0 + s0 // 16:c0 + (s0 + ns) // 16],
                                ns, nidx_reg(ns), re_in, elem_step=re_in,
                            )
                        for tt in range(ntile):
                            t = t0 + tt
                            pv = spool.tile([128, m_ovf, 128], BF, tag="povf")
                            pv_src = povf_d[k][t, :, :, :].rearrange(
                                "b p n -> p b n")
                            nc.sync.dma_start(pv[:], pv_src)
                            agg = psA.tile([128, 512], F32, tag="agg")
                            base = tt * gpt
                            # each 32-row quarter is started by its first
                            # matmul; identity/overflow then accumulate over
                            # the whole tile and the last one stops the group
                            for qg in range(16):
                                q, g = qg // 4, qg % 4
                                nc.tensor.matmul(
                                    agg[q * 32:(q + 1) * 32, 0:fin], p32_sb[:, :],
                                    gbuf[:, base + qg, 0:fin],
                                    start=(g == 0), stop=False,
                                    skip_group_check=True,
                                    tile_position=(0, q * 32))
                            for b in range(m_ovf):
                                nc.tensor.matmul(
                                    agg[:, 0:fin], pv[:, b, :],
                                    gbuf[:, base + 16 + b, 0:fin],
                                    start=False, stop=(b == m_ovf - 1),
                                    skip_group_check=True)
                            # s' = [scale*agg, col]
                            sp = wpool.tile([128, fin + 1], BF, tag="sp")
                            scale = dinv2l[k] if lay < 2 else dinvl[k]
                            nc.scalar.activation(
                                sp[:, 0:fin], agg[:, 0:fin], AF.Copy,
                                scale=scale[:, t:t + 1])
                            col = dinvlbf[k][:, t:t + 1] if lay < 2 \
                                else onesbf[:, 0:1]
                            nc.vector.tensor_copy(sp[:, fin:fin + 1], col)
                            # transpose s' -> [fin+1, 128]
                            spT = psB.tile([128, 1024], BF, tag="spT")
                            if fin + 1 <= 128:
                                nc.tensor.transpose(
                                    spT[0:fin + 1, 0:128], sp[:, :], ident_sb[:, :])
                            else:
                                nc.tensor.transpose(
                                    spT[:, 0:128], sp[:, 0:128], ident_sb[:, :])
                                spT2 = psC.tile([32, 1024], BF, tag="spT2")
                                nc.tensor.transpose(
                                    spT2[0:fin + 1 - 128, 0:128],
                                    sp[:, 128:fin + 1], ident_sb[:, :])
                            if lay < 2:
                                spT_sb = wpool.tile([128, 128], BF, tag="spT_sb")
                                nc.vector.tensor_copy(
                                    spT_sb[0:fin + 1, :], spT[0:fin + 1, 0:128])
                                h = psA.tile([128, 512], F32, tag="h")
                                (w_t, _), = wsb[f"w{lay + 1}"]
                                nc.tensor.matmul(
                                    h[:, 0:fout], spT_sb[0:fin + 1, :],
                                    w_t[0:fin + 1, 0:fout],
                                    start=True, stop=True)
                                slab_sb = wpool.tile([128, fout], BF, tag="slab")
                                nc.scalar.activation(
                                    slab_sb[:], h[:, 0:fout], AF.Relu)
                                slab_d = [slab2, slab3][lay][k]
                                nc.sync.dma_start(
                                    slab_d[t * 128:(t + 1) * 128, 0:fout],
                                    slab_sb[:])
                            else:
                                nc.vector.tensor_copy(
                                    s3a[:, t * 128:(t + 1) * 128], spT[:, 0:128])
                                nc.vector.tensor_copy(
                                    s3b[0:F2 + 1 - 128, t * 128:(t + 1) * 128],
                                    spT2[0:F2 + 1 - 128, 0:128])
                    if lay < 2:
                        slab_d = [slab2, slab3][lay][k]
                        tab_out = [tab2, tab3][lay][k]
                        nc.gpsimd.collective_compute(
                            "AllGather", ALU.bypass, replica_groups=RG,
                            ins=[slab_d[:, :]],
                            outs=[tab_out[0:N_NODES, :]],
                        )

            for k in range(2):
                s3a, s3b = s3ab[k]
                # ---- h3T = relu(W3'.T @ s3T), feature-major ----
                w3a, w3b = wsb["w3"][0][0], wsb["w3"][1][0]
                h3T = [bpool.tile([128, NLOC], BF, tag=f"h3T{i}", name=f"h3T{i}_{k}") for i in range(3)]
                fo_sz = [128, 128, F3 - 256]
                for fo in range(3):
                    for nb in range(0, NLOC, 512):
                        nbs = min(512, NLOC - nb)
                        h3 = psA.tile([128, 512], F32, tag="h")
                        nc.tensor.matmul(
                            h3[0:fo_sz[fo], 0:nbs],
                            w3a[0:128, fo * 128:fo * 128 + fo_sz[fo]],
                            s3a[:, nb:nb + nbs], start=True, stop=False,
                            skip_group_check=True)
                        nc.tensor.matmul(
                            h3[0:fo_sz[fo], 0:nbs],
                            w3b[0:F2 + 1 - 128, fo * 128:fo * 128 + fo_sz[fo]],
                            s3b[0:F2 + 1 - 128, nb:nb + nbs],
                            start=False, stop=True,
                            skip_group_check=True)
                        nc.scalar.activation(
                            h3T[fo][0:fo_sz[fo], nb:nb + nbs],
                            h3[0:fo_sz[fo], 0:nbs], AF.Relu)
                # ---- segment max pool: [fo, 64 graphs] ----
                pooledT = []
                for fo in range(3):
                    pt = wpool.tile([128, GLOC], BF, tag=f"pooledT{fo}", name=f"pooledT{fo}_{k}")
                    src3 = h3T[fo][0:fo_sz[fo], :].rearrange(
                        "p (g w) -> p g w", w=NPG)
                    nc.vector.reduce_max(
                        pt[0:fo_sz[fo], :], src3, axis=mybir.AxisListType.X)
                    pooledT.append((pt, fo_sz[fo]))
                pooledT.append((ones64, 1))

                # ---- fc_g1 (relu) + fc_g2 ----
                fg1 = dense_T(pooledT, "wg1", "relu", f"g1_{k}")
                fg1.append((ones64, 1))
                dk = dense_T(fg1, "wg2", "none", f"g2_{k}")
                dT.append(dk[0][0])

            # ---------------- head ----------------
            cell_sb = spool.tile([128, FXT], F32, tag="cell")
            nc.sync.dma_start(cell_sb[0:GLOC, :], cell_d[:])
            sq = wpool.tile([128, FXT], F32, tag="cellsq")
            nc.vector.tensor_mul(sq[0:GLOC, :], cell_sb[0:GLOC, :],
                                 cell_sb[0:GLOC, :])
            ss = wpool.tile([128, 1], F32, tag="cellss")
            nc.vector.reduce_sum(ss[0:GLOC, :], sq[0:GLOC, :],
                                 axis=mybir.AxisListType.X)
            nc.scalar.sqrt(ss[0:GLOC, :], ss[0:GLOC, :])
            nc.vector.tensor_scalar_max(ss[0:GLOC, :], ss[0:GLOC, :], 1e-12)
            rn = wpool.tile([128, 1], F32, tag="cellrn")
            nc.vector.reciprocal(rn[0:GLOC, :], ss[0:GLOC, :])
            cvn = wpool.tile([128, FXT], BF, tag="cvn")
            nc.vector.tensor_scalar_mul(cvn[0:GLOC, :], cell_sb[0:GLOC, :],
                                        rn[0:GLOC, :])
            # transpose cvn -> cellT chunks
            cellT = []
            r0 = 0
            while r0 < FXT:
                r1 = min(r0 + 128, FXT)
                cps = psC.tile([128, 1024], BF, tag="hps")
                nc.tensor.transpose(
                    cps[0:r1 - r0, 0:GLOC], cvn[0:GLOC, r0:r1],
                    ident_sb[0:GLOC, 0:GLOC])
                ct = wpool.tile([128, GLOC], BF, tag=f"cellT{r0}", name=f"cellT{r0}")
                nc.vector.tensor_copy(ct[0:r1 - r0, :], cps[0:r1 - r0, 0:GLOC])
                cellT.append((ct, r1 - r0))
                r0 = r1
            cellT.append((ones64, 1))

            def with_ones(chunks):
                return list(chunks) + [(ones64, 1)]

            r1o = dense_T(cellT, "wr1", "relu", "r1")
            r2o = dense_T(with_ones(r1o), "wr2", "relu", "r2")
            cvo = dense_T(with_ones(r2o), "wr3", "none", "r3")
            gate = dense_T([(dT[0], 128), (dT[1], 128), (ones64, 1)],
                           "wgt", "sigmoid", "gt")
            xc = [(dT[0], 128), (dT[1], 128), (gate[0][0], 128),
                  (cvo[0][0], 128), (ones64, 1)]
            f1o = dense_T(xc, "wf1", "relu", "f1")
            f2o = dense_T(with_ones(f1o), "wf2", "relu", "f2")
            oo = dense_T(with_ones(f2o), "wo", "none", "oo")
            out_sb = wpool.tile([128, GLOC], F32, tag="outsb")
            nc.vector.tensor_copy(out_sb[0:N_OUT, :], oo[0][0][0:N_OUT, :])
            nc.sync.dma_start(out_d[:, :], out_sb[0:N_OUT, :])

    nc.compile()
    return nc


# dense_T writes bf16 outputs; the final "oo" needs f32 — handled by copying
# the bf16 tile to f32 out_sb above (acceptable: last layer output rounding).


# ===========================================================================
# Entry point
# ===========================================================================

_CACHE = {}


INPUT_NAMES = ("x1_full", "x2_full", "deg1_full", "deg2_full", "deg1_loc",
               "deg2_loc", "idx1", "idx2", "povf1", "povf2", "cell_loc",
               "ident", "p32", "w1", "w2", "w3", "wg1", "wg2", "wr1", "wr2",
               "wr3", "wgt", "wf1", "wf2", "wo")


def run_device(meta, per_core, **kwargs):
    """Compile (cached) + run on the 8 NeuronCores. Returns (out, results)."""
    from concourse.bass_utils import run_bass_kernel_spmd

    key = meta["m_ovf"]
    if key not in _CACHE:
        _CACHE[key] = build_bass(meta)
    nc = _CACHE[key]

    in_maps = [{n: per_core[c][n] for n in INPUT_NAMES} for c in range(NCORES)]
    res = run_bass_kernel_spmd(nc, in_maps, list(range(NCORES)), **kwargs)
    outs = [np.asarray(res.results[c]["outT"]) for c in range(NCORES)]
    out = np.concatenate([o.T for o in outs], axis=0).astype(np.float32)
    return out, res


def kernel(x1, edge_index1, batch1, x2, edge_index2, batch2, cell, params):
    meta, per_core = prep_all(x1, edge_index1, x2, edge_index2, cell, params)
    out, _ = run_device(meta, per_core)
    return out
